# revision 15
# baseline (speedup 1.0000x reference)
"""Multi-head causal self-attention (GPT-style block) on 8 Trainium2 NeuronCores.

Data-parallel over batch (B=8 -> 1 element/core), weights replicated.

v2 design (vs v1 baseline ~188us):
- Bias algebra: k-bias dropped (softmax row-shift invariance), v-bias folded
  into b_proj on host, q-bias folded EXACTLY into per-key factors
  et[k] = exp(0.125 * bq_h . k_h[k]) multiplied into V and the denominator
  ("ones") column. Scores then need no bias -> wide exp instructions, plain
  copy evacuations.
- Scores matmuls are K=64 and run CONCURRENTLY per head pair via PE row
  tiling (lhsT/rhs at partition bases 0/64 -> tile_position (0,0)/(64,0)).
- ScalarE: exp + a few early evacs; DVE: psum evacs/normalize; Pool: masks.
- qg0+qg1 attention interleaved per head pair with qkv/proj matmuls as PE
  filler; PE warm-up junk matmuls + multi-queue DMA issue shrink the head.
"""

import numpy as np

import concourse.bass as bass
import concourse.mybir as mybir
import concourse.tile as tile
from concourse import bacc, bass_utils
from concourse.masks import make_identity, make_upper_triangular

F32 = mybir.dt.float32
BF16 = mybir.dt.bfloat16
EXP = mybir.ActivationFunctionType.Exp

T = 1024
H = 768
NH = 12
HS = 64
TT = T // 128   # 8 token tiles
FT = H // 128   # 6 feature tiles
NP = NH // 2    # 6 head pairs
N_CORES = 8

PAIRS = {0: [(0, 1), (2, 3)], 1: [(0, 1), (2, 3), (4, 5), (6, 7)]}


def _pair_geom(qg, kt0, kt1):
    off0 = max(128 * kt0, 512 * qg)
    off1 = max(128 * kt1, 512 * qg)
    return off0, 512 * (qg + 1) - off0, off1, 512 * (qg + 1) - off1


def build():
    nc = bacc.Bacc(None, target_bir_lowering=False)

    x_d = nc.dram_tensor("x", [T, H], BF16, kind="ExternalInput")
    wa_d = nc.dram_tensor("W_attn", [H, 3 * H], BF16, kind="ExternalInput")
    bq_d = nc.dram_tensor("bq", [128, NP], BF16, kind="ExternalInput")
    wp_d = nc.dram_tensor("W_proj", [H, H], BF16, kind="ExternalInput")
    bp_d = nc.dram_tensor("bp", [H], BF16, kind="ExternalInput")
    y_d = nc.dram_tensor("y", [T, H], F32, kind="ExternalOutput")

    with tile.TileContext(nc) as tc:
        with (
            tc.tile_pool(name="sb", bufs=1) as sb,
            tc.tile_pool(name="ps", bufs=1, space="PSUM") as ps,
        ):
            # ---------------- persistent SBUF ----------------
            wat = sb.tile([128, FT, 3 * H], BF16, tag="wat")
            wpr = sb.tile([128, FT, H], BF16, tag="wpr")
            x_bf = sb.tile([128, TT, H], BF16, tag="x_bf")
            xT = sb.tile([128, FT, T], BF16, tag="xT")
            qT = sb.tile([128, NP, T], BF16, tag="qT")
            kT = sb.tile([128, NP, T], BF16, tag="kT")
            v_pl = sb.tile([128, TT, H], BF16, tag="v_pl")       # v (no et)
            v_bf = sb.tile([128, TT, NH * (HS + 1) + 64], BF16, tag="v_bf")
            oT = sb.tile([128, FT, T], BF16, tag="oT")
            etT = sb.tile([128, NP, 2, TT], BF16, tag="etT")
            bqc = sb.tile([128, NP], BF16, tag="bqc")
            bp_rowb = sb.tile([1, H], BF16, tag="bp_rowb")
            ones0 = sb.tile([1, 128], BF16, tag="ones0")
            ones64 = sb.tile([65, 128], BF16, tag="ones64")
            tri = sb.tile([128, 128], BF16, tag="tri")
            ident = sb.tile([128, 128], BF16, tag="ident")
            warm = sb.tile([128, 512], BF16, tag="warm")
            junks = sb.tile([1, 16], F32, tag="junks")

            # ---------------- consts (pool engine, front of its queue) ----
            make_identity(nc, ident[:])
            make_upper_triangular(nc, tri[:], val=1.0, diag=True)
            nc.gpsimd.memset(warm[:], 0.125)
            nc.gpsimd.memset(ones0[:], 1.0)
            nc.gpsimd.memset(ones64[64:65, :], 1.0)
            nc.gpsimd.memset(v_bf[:, :, NH * (HS + 1):], 0.0)

            # ---------------- PE warm-up (HAM) while DMAs land ------------
            with nc.named_scope("head"):
                pw = ps.tile([128, 512], F32, tag="av", bufs=2, name="pw")
                for _ in range(14):
                    nc.tensor.matmul(pw[:], warm[:, :128], warm[:], start=True, stop=True)
                # ACT exp table preload
                nc.scalar.activation(junks[:], warm[:1, :16], EXP, scale=0.125)

            # ---------------- DMA issue, spread across queues -------------
            for tt in range(TT):
                nc.sync.dma_start(x_bf[:, tt, :], x_d[tt * 128:(tt + 1) * 128, :])
            for ft in range(FT):
                eng = nc.scalar if ft < 3 else nc.sync
                eng.dma_start(wat[:, ft, :2 * H], wa_d[ft * 128:(ft + 1) * 128, :2 * H])
            nc.gpsimd.dma_start(bqc[:], bq_d[:, :])
            for ft in range(FT):
                nc.gpsimd.dma_start(wat[:, ft, 2 * H:], wa_d[ft * 128:(ft + 1) * 128, 2 * H:])
            for ft in range(FT):
                nc.gpsimd.dma_start(wpr[:, ft, :], wp_d[ft * 128:(ft + 1) * 128, :])
            nc.gpsimd.dma_start(bp_rowb[:], bp_d[None, :])

            # ---------------- x transpose (evac on ACT: idle pre-attn) ----
            with nc.named_scope("xT"):
                for tt in range(TT):
                    pt = ps.tile([128, FT * 128], BF16, tag="op", bufs=2, name="pt")
                    for ft in range(FT):
                        nc.tensor.transpose(
                            pt[:, ft * 128:(ft + 1) * 128],
                            x_bf[:, tt, ft * 128:(ft + 1) * 128],
                            ident[:],
                        )
                    nc.scalar.copy(
                        xT[:, :, tt * 128:(tt + 1) * 128],
                        pt[:].rearrange("p (f t) -> p f t", t=128),
                    )

            # ---------------- emission helpers ----------------------------
            def emit_qk(hp, which):
                """q (nt=hp) or k (nt=6+hp) projection -> qT/kT, DVE evac."""
                nt, dst = (hp, qT) if which == "q" else (NP + hp, kT)
                for tg in range(2):
                    pq = ps.tile([128, 512], F32, tag="op", bufs=2, name="pq")
                    for ft in range(FT):
                        nc.tensor.matmul(
                            pq[:],
                            wat[:, ft, nt * 128:(nt + 1) * 128],
                            xT[:, ft, tg * 512:(tg + 1) * 512],
                            start=(ft == 0),
                            stop=(ft == FT - 1),
                        )
                    nc.vector.tensor_copy(dst[:, hp, tg * 512:(tg + 1) * 512], pq[:])

            def emit_t(hp):
                """et = exp(0.125*bq_h . k_h) for both heads of the pair."""
                etp = ps.tile([128, 512], F32, tag="op", bufs=2, name="etp")
                for hi in range(2):
                    for kt in range(TT):
                        nc.tensor.matmul(
                            etp[:, hi * 8 + kt: hi * 8 + kt + 1],
                            kT[hi * 64:(hi + 1) * 64, hp, kt * 128:(kt + 1) * 128],
                            bqc[hi * 64:(hi + 1) * 64, hp:hp + 1],
                            start=True,
                            stop=True,
                        )
                nc.scalar.activation(
                    etT[:, hp].rearrange("p h t -> p (h t)"), etp[:, :16],
                    EXP, scale=0.125,
                )

            def emit_vmm(tt, early):
                """v projection for one token tile -> v_pl."""
                for ng in range(2):
                    pv = ps.tile([128, 512], F32, tag="op", bufs=2, name="pv")
                    for ft in range(FT):
                        nc.tensor.matmul(
                            pv[:, :384],
                            xT[:, ft, tt * 128:(tt + 1) * 128],
                            wat[:, ft, 2 * H + 384 * ng: 2 * H + 384 * (ng + 1)],
                            start=(ft == 0),
                            stop=(ft == FT - 1),
                        )
                    # early tiles: ACT is idle pre-attn; later ones: DVE
                    eng = nc.scalar if early else nc.vector
                    if early:
                        eng.copy(v_pl[:, tt, 384 * ng:384 * (ng + 1)], pv[:, :384])
                    else:
                        eng.tensor_copy(v_pl[:, tt, 384 * ng:384 * (ng + 1)], pv[:, :384])

            def emit_etapply(hp, half):
                """v_bf[:, tts, pair slots] = v_pl * et ; slot 64 = et."""
                ts = slice(4 * half, 4 * half + 4)
                dst3 = v_bf[:, ts, 130 * hp:130 * (hp + 1)].rearrange(
                    "p t (h d) -> p t h d", d=65)
                src3 = v_pl[:, ts, 128 * hp:128 * (hp + 1)].rearrange(
                    "p t (h d) -> p t h d", d=64)
                etr = etT[:, hp, :, ts].rearrange("p h t -> p t h")
                et4 = etr.unsqueeze(3).broadcast_to((128, 4, 2, 64))
                nc.vector.tensor_mul(dst3[:, :, :, :64], src3[:], et4)
                nc.vector.tensor_copy(dst3[:, :, :, 64:65], etr.unsqueeze(3))

            def emit_scores_pair(hp, qg, pi, pts):
                """scores + exp + mask for one kt pair."""
                kt0, kt1 = PAIRS[qg][pi]
                off0, w0, off1, w1 = _pair_geom(qg, kt0, kt1)
                sps = [
                    ps.tile([128, 1024], F32, tag=f"s{hi}", bufs=1, name=f"sp{hi}")
                    for hi in range(2)
                ]
                for kt, off, w, so in ((kt0, off0, w0, 0), (kt1, off1, w1, w0)):
                    for hi in range(2):
                        nc.tensor.matmul(
                            sps[hi][:, so:so + w],
                            kT[hi * 64:(hi + 1) * 64, hp, kt * 128:(kt + 1) * 128],
                            qT[hi * 64:(hi + 1) * 64, hp, off:off + w],
                            start=True,
                            stop=True,
                        )
                vw = w0 + w1
                diag = 128 * kt0 >= 512 * qg
                for hi in range(2):
                    nc.scalar.activation(
                        pts[hi][:, pi, :vw], sps[hi][:, :vw], EXP, scale=0.125)
                    if diag:
                        nc.gpsimd.tensor_mul(
                            pts[hi][:, pi, :128], pts[hi][:, pi, :128], tri[:])
                        nc.gpsimd.tensor_mul(
                            pts[hi][:, pi, w0:w0 + 128], pts[hi][:, pi, w0:w0 + 128], tri[:])

            def emit_av(hp, qg, pts, avs, pis, first, last):
                """AV accumulation for pair indices pis of (hp, qg)."""
                npairs = len(PAIRS[qg])
                for hi in range(2):
                    h = 2 * hp + hi
                    for pi in pis:
                        kt0, kt1 = PAIRS[qg][pi]
                        off0, w0, off1, w1 = _pair_geom(qg, kt0, kt1)
                        for kt, off, w, so in ((kt0, off0, w0, 0), (kt1, off1, w1, w0)):
                            nc.tensor.matmul(
                                avs[hi][:, off - 512 * qg: off - 512 * qg + w],
                                v_bf[:, kt, 65 * h:65 * h + 128],
                                pts[hi][:, pi, so:so + w],
                                start=(first and pi == pis[0] and so == 0),
                                stop=(last and pi == pis[-1] and so == w0),
                            )

            def emit_recip(avs):
                recbs = []
                for hi in range(2):
                    rec = sb.tile([65, 512], F32, tag="rec", bufs=3, name="rec")
                    recb = sb.tile([65, 512], BF16, tag="recb", bufs=3, name="recb")
                    nc.vector.reciprocal_approx_fast(rec[:, :], avs[hi][:65, :])
                    nc.vector.tensor_copy(recb[64:65, :], rec[64:65, :])
                    recbs.append(recb)
                return recbs

            def emit_norm(hp, qg, avs, recbs, hi):
                bp = ps.tile([128, 512], F32, tag="op", bufs=2, name="bp")
                nc.tensor.matmul(
                    bp[:64, :], ones64[64:65, :64], recbs[hi][64:65, :],
                    start=True, stop=True,
                )
                bpb = sb.tile([64, 512], BF16, tag="bpb", bufs=4, name="bpb")
                nc.vector.tensor_copy(bpb[:], bp[:64, :])
                dst = slice(512 * qg, 512 * (qg + 1))
                if hi == 0:
                    nc.vector.tensor_mul(oT[:64, hp, dst], avs[0][:64, :], bpb[:])
                else:
                    sc = sb.tile([64, 512], BF16, tag="sc", bufs=4, name="sc")
                    nc.vector.tensor_mul(sc[:], avs[1][:64, :], bpb[:])
                    nc.sync.dma_start(oT[64:128, hp, dst], sc[:])

            def emit_proj(tt):
                ysb = sb.tile([128, H], F32, tag="ysb", bufs=4, name="ysb")
                for ng in range(2):
                    py = ps.tile([128, 512], F32, tag="op", bufs=2, name="py")
                    for ft in range(FT):
                        nc.tensor.matmul(
                            py[:, :384],
                            oT[:, ft, tt * 128:(tt + 1) * 128],
                            wpr[:, ft, 384 * ng:384 * (ng + 1)],
                            start=(ft == 0),
                            stop=False,
                        )
                    nc.tensor.matmul(
                        py[:, :384],
                        ones0[:1, :],
                        bp_rowb[:1, 384 * ng:384 * (ng + 1)],
                        start=False,
                        stop=True,
                    )
                    nc.vector.tensor_copy(ysb[:, 384 * ng:384 * (ng + 1)], py[:, :384])
                nc.sync.dma_start(y_d[tt * 128:(tt + 1) * 128, :], ysb[:])

            # ---------------- main pipeline -------------------------------
            # step hp: scores(hp) both qg, AV/recip/norm(hp-1), interleaved
            # with qk/t/v/etapply fillers between dependent chunks.
            with nc.named_scope("attn"):
                emit_qk(0, "q")
                emit_qk(0, "k")
                emit_t(0)
                for tt in range(4):
                    emit_vmm(tt, early=True)
                emit_etapply(0, 0)

                state = {}
                for hp in range(NP + 1):
                    prev = state.get(hp - 1)
                    if prev is not None:
                        prev["av0"] = [
                            ps.tile([128, 512], F32, tag="av", bufs=2, name="av0")
                            for _ in range(2)
                        ]
                    if hp < NP:
                        pts0 = [
                            sb.tile([128, 2, 1024], BF16, tag=f"pA{hi}", bufs=2,
                                    name=f"ptsA{hi}")
                            for hi in range(2)
                        ]
                        pts = [
                            sb.tile([128, 4, 1024], BF16, tag=f"p{hi}", bufs=2,
                                    name=f"pts{hi}")
                            for hi in range(2)
                        ]
                        state[hp] = {"pts": pts, "pts0": pts0}
                        emit_scores_pair(hp, 0, 0, pts0)
                        if prev is not None:
                            emit_av(hp - 1, 0, prev["pts0"], prev["av0"], [0],
                                    first=True, last=False)
                        emit_scores_pair(hp, 0, 1, pts0)
                        if prev is not None:
                            emit_av(hp - 1, 0, prev["pts0"], prev["av0"], [1],
                                    first=False, last=True)
                            prev["r0"] = emit_recip(prev["av0"])
                        emit_scores_pair(hp, 1, 0, pts)
                        if hp == 0:
                            for tt in range(4, TT):
                                emit_vmm(tt, early=False)
                            emit_etapply(0, 1)
                        if hp + 1 < NP:
                            emit_qk(hp + 1, "q")
                        if prev is not None:
                            emit_norm(hp - 1, 0, prev["av0"], prev["r0"], 0)
                            emit_norm(hp - 1, 0, prev["av0"], prev["r0"], 1)
                        emit_scores_pair(hp, 1, 1, pts)
                        if prev is not None:
                            prev["av1"] = [
                                ps.tile([128, 512], F32, tag="av", bufs=2, name="av1")
                                for _ in range(2)
                            ]
                            emit_av(hp - 1, 1, prev["pts"], prev["av1"], [0, 1],
                                    first=True, last=False)
                        emit_scores_pair(hp, 1, 2, pts)
                        if hp + 1 < NP:
                            emit_qk(hp + 1, "k")
                            emit_t(hp + 1)
                        emit_scores_pair(hp, 1, 3, pts)
                        if prev is not None:
                            emit_av(hp - 1, 1, prev["pts"], prev["av1"], [2, 3],
                                    first=False, last=True)
                            prev["r1"] = emit_recip(prev["av1"])
                        if hp + 1 < NP:
                            emit_etapply(hp + 1, 0)
                            emit_etapply(hp + 1, 1)
                        if prev is not None:
                            emit_norm(hp - 1, 1, prev["av1"], prev["r1"], 0)
                            emit_norm(hp - 1, 1, prev["av1"], prev["r1"], 1)
                    else:
                        # flush last head pair
                        emit_av(hp - 1, 0, prev["pts0"], prev["av0"], [0, 1],
                                first=True, last=True)
                        prev["r0"] = emit_recip(prev["av0"])
                        prev["av1"] = [
                            ps.tile([128, 512], F32, tag="av", bufs=2, name="av1")
                            for _ in range(2)
                        ]
                        emit_av(hp - 1, 1, prev["pts"], prev["av1"], [0, 1],
                                first=True, last=False)
                        emit_norm(hp - 1, 0, prev["av0"], prev["r0"], 0)
                        emit_norm(hp - 1, 0, prev["av0"], prev["r0"], 1)
                        emit_av(hp - 1, 1, prev["pts"], prev["av1"], [2, 3],
                                first=False, last=True)
                        prev["r1"] = emit_recip(prev["av1"])
                        emit_proj(0)
                        emit_norm(hp - 1, 1, prev["av1"], prev["r1"], 0)
                        emit_norm(hp - 1, 1, prev["av1"], prev["r1"], 1)
                        emit_proj(1)

            # ---------------- output projection ---------------------------
            with nc.named_scope("proj"):
                for tt in range(2, TT):
                    emit_proj(tt)

    nc.compile()
    return nc


_NC = None


def _run(in_maps, trace=False, **kwargs):
    global _NC
    if _NC is None:
        _NC = build()
    return bass_utils.run_bass_kernel_spmd(
        _NC, in_maps, core_ids=list(range(N_CORES)), trace=trace, **kwargs
    )


def make_in_maps(x, W_attn, b_attn, W_proj, b_proj):
    import ml_dtypes
    bf = ml_dtypes.bfloat16
    x = np.asarray(x, dtype=np.float32).astype(bf)
    W_attn_f = np.asarray(W_attn, dtype=np.float32)
    b_attn_f = np.asarray(b_attn, dtype=np.float32)
    W_proj_f = np.asarray(W_proj, dtype=np.float32)
    b_proj_f = np.asarray(b_proj, dtype=np.float32)

    W_attn_b = np.ascontiguousarray(W_attn_f.astype(bf))
    W_proj_b = np.ascontiguousarray(W_proj_f.astype(bf))
    # q-bias as per-partition columns: col hp = [bq[2hp] | bq[2hp+1]]
    bq = b_attn_f[:H].reshape(NH, HS)
    bqc = np.empty((128, NP), dtype=np.float32)
    for hp in range(NP):
        bqc[:64, hp] = bq[2 * hp]
        bqc[64:, hp] = bq[2 * hp + 1]
    bqc = np.ascontiguousarray(bqc.astype(bf))
    # v-bias folded into projection bias (exact: sum_k P = 1 per row)
    bp_eff = np.ascontiguousarray(
        (b_attn_f[2 * H:] @ W_proj_f + b_proj_f).astype(bf))
    return [
        {
            "x": np.ascontiguousarray(x[b]),
            "W_attn": W_attn_b,
            "bq": bqc,
            "W_proj": W_proj_b,
            "bp": bp_eff,
        }
        for b in range(N_CORES)
    ]


def kernel(x, W_attn, b_attn, W_proj, b_proj):
    in_maps = make_in_maps(x, W_attn, b_attn, W_proj, b_proj)
    res = _run(in_maps, trace=False)
    return np.stack([res.results[b]["y"] for b in range(N_CORES)]).astype(np.float32)
